# revision 11
# baseline (speedup 1.0000x reference)
"""Trainium2 Bass kernel for nn_AudioLSTM (2-layer LSTM + 2-layer FC head).

Strategy (per core; pure data parallelism over batch, 8 cores x 64 batch):
  - Two independent batch halves of 32 (software pipelining of the serial
    T=1000 recurrence across engines).
  - Sliding-window "strip" state layout: strip[123, 250*64] bf16 per chunk
    (double buffered).  Rows 0:96 = [H1(64); H2(32)] (H = 2*h), rows
    96:122 = x_t, row 122 = ones.  Block t (cols t*64:(t+1)*64) is the
    matmul rhs for step t; the H update of step t writes directly into
    block t+1, and x (+ ones row) is staged by one contiguous DMA per
    250-step chunk from a host-side pre-transposed [27, T, 64] bf16 tensor.
    => zero per-step data-movement ops.
  - 4 matmuls per step per half (one per gate q in {i,f,o,g}), each
    lhsT [123, MW] -> psum [MW, 32]: rows 0:64 = LSTM1 gate, 64:96 = LSTM2
    gate (MW=128 pads gate blocks with zero cols to trigger FWL).
    LSTM2 runs one step behind LSTM1 so both layers read the same block.
  - tanh-everywhere: sigma(z) = (1+tanh(z/2))/2; the 1/2 scales are folded
    into the packed weights, so ONE Tanh activation covers all 4 gates.
  - Cell state C = 2*c in fp32:  A=(tf+1)*C; B=(ti+1)*tg; C=0.5*A+B;
    th=tanh(0.5*C); H=(to+1)*th   (4 DVE scalar_tensor_tensor ops).
  - Iteration 0 uses a weight copy with the LSTM2 columns zeroed, which makes
    the one-step-behind LSTM2 start exactly from h2=c2=0.  One extra
    iteration (t=T) lets LSTM2 finish its last step.
"""
import os
import sys
from contextlib import ExitStack

import numpy as np

sys.path.insert(0, "/opt/trn_rl_repo")

import ml_dtypes

import concourse.bacc as bacc
import concourse.mybir as mybir
from concourse import bass_utils, tile

AF = mybir.ActivationFunctionType
ALU = mybir.AluOpType
BF16 = mybir.dt.bfloat16
F16 = mybir.dt.float16
F32 = mybir.dt.float32

IN, H1, H2, F1, OUT = 26, 64, 32, 16, 10
B, T = 512, 1000
NCORES = 8
BL = B // NCORES          # 64 batch per core
NH = 2                    # batch halves per core (software pipelining)
BH = BL // NH             # 32
TC = 250                  # time chunk for x staging
NCK = T // TC
KP = 123                  # state rows: 96 H + 26 x + 1 ones


def _build_body(ctx: ExitStack, tc_: tile.TileContext, x, w, w0, wfc1, wfc2, out,
                mw=128, f16=True):
    nc = tc_.nc
    DT = F16 if f16 else F32

    const = ctx.enter_context(tc_.tile_pool(name="const", bufs=1))
    psum = ctx.enter_context(tc_.tile_pool(name="ps", bufs=3, space="PSUM"))
    work = ctx.enter_context(tc_.tile_pool(name="wk", bufs=4))

    w_sb = const.tile([KP, 4 * mw], BF16)
    nc.sync.dma_start(out=w_sb, in_=w)
    w0_sb = const.tile([KP, 4 * mw], BF16)
    nc.sync.dma_start(out=w0_sb, in_=w0)
    wfc1_sb = const.tile([33, F1], BF16)
    nc.sync.dma_start(out=wfc1_sb, in_=wfc1)
    wfc2_sb = const.tile([33, OUT], BF16)
    nc.sync.dma_start(out=wfc2_sb, in_=wfc2)

    # Pre-warm the ACT tanh table during startup DMAs so the first real TANH
    # doesn't pay the ~2.7us ACT_TABLE_LOAD on the critical path.
    warm = work.tile([1, 1], F32)
    nc.any.memset(warm, 0.0)
    wout = work.tile([1, 1], F32)
    nc.scalar.activation(wout, warm, AF.Tanh)

    strips = [const.tile([KP, BL * TC], BF16, name=f"strip{i}") for i in range(2)]
    # chunk 0 x (+ones row) staging; split so the first steps' x lands fast
    # and the recurrence starts without waiting on the full 864KB transfer.
    PRE = 16
    nc.sync.dma_start(out=strips[0][96:123, 0:PRE * BL], in_=x[:, 0:PRE, :])
    nc.sync.dma_start(out=strips[0][96:123, PRE * BL:], in_=x[:, PRE:TC, :])
    nc.vector.memset(strips[0][0:96, 0:BL], 0.0)
    # Cell-state tiles live in the work-pool arena (bufs=1 -> persistent slot)
    # so the per-step DVE ops touching them stay within one SBUF neighborhood;
    # const-pool placement behind the 62KB strips measured ~80ns/op slower.
    Cs = []
    for h in range(NH):
        C_h = work.tile([96, BH], DT, name=f"C{h}", tag=f"C{h}", bufs=1)
        nc.any.memset(C_h, 0.0)
        Cs.append(C_h)
    # Near-placed zero bias vector for the activations (the implicit const-AP
    # bias lives in a far SBUF region).
    zbias = work.tile([96, 1], F32, name="zbias", tag="zbias", bufs=1)
    nc.any.memset(zbias, 0.0)

    out_sb = const.tile([OUT, BL], F32)

    for t in range(T + 1):
        ck, tt = divmod(t, TC)
        buf = strips[ck % 2]
        col = tt * BL
        if tt == 0 and ck + 1 < NCK:
            nxt = strips[(ck + 1) % 2]
            nc.sync.dma_start(
                out=nxt[96:123, :], in_=x[:, (ck + 1) * TC:(ck + 2) * TC, :]
            )
        ck2, tt2 = divmod(t + 1, TC)
        buf2 = strips[ck2 % 2]
        col2 = tt2 * BL
        wsel = w0_sb if t == 0 else w_sb
        # Emission order is engine-queue order (in-order engines).  Interleave
        # the two halves' chains so ACT runs TANH(h0), TANH(h1), T05(h0),
        # T05(h1) per step instead of serializing each half's full chain.
        ASs = []
        for h in range(NH):
            rhs = buf[:, col + h * BH:col + (h + 1) * BH]
            ps = psum.tile([mw, 4 * BH], F32, name="ps", tag=f"ps{h}")
            for gi in range(4):
                nc.tensor.matmul(
                    ps[:, gi * BH:(gi + 1) * BH],
                    wsel[:, gi * mw:(gi + 1) * mw],
                    rhs,
                    start=True,
                    stop=True,
                )
            AS = work.tile([96, 4 * BH], DT, name="AS", tag=f"AS{h}")
            nc.scalar.activation(AS, ps[0:96, :], AF.Tanh, bias=zbias)
            ASs.append(AS)
        # Interleave both halves' product ops before the C updates: the DVE
        # stalls ~100ns when an op reads the immediately-preceding op's output
        # (pipe-drain RAW); with Bv/Av of both halves first, each C's inputs
        # are >=2 ops back and the drain is hidden.
        AvBv = []
        for h in range(NH):
            AS = ASs[h]
            ti = AS[:, 0:BH]
            tf = AS[:, BH:2 * BH]
            tg = AS[:, 3 * BH:4 * BH]
            Bv = work.tile([96, BH], DT, name="Bv", tag=f"Bv{h}")
            nc.vector.scalar_tensor_tensor(Bv, ti, 1.0, tg, ALU.add, ALU.mult)
            Av = work.tile([96, BH], DT, name="Av", tag=f"Av{h}")
            nc.vector.scalar_tensor_tensor(Av, tf, 1.0, Cs[h], ALU.add, ALU.mult)
            AvBv.append((Av, Bv))
        for h in range(NH):
            Av, Bv = AvBv[h]
            nc.vector.scalar_tensor_tensor(Cs[h], Av, 0.5, Bv, ALU.mult, ALU.add)
        for h in range(NH):
            to = ASs[h][:, 2 * BH:3 * BH]
            th = work.tile([96, BH], DT, name="th", tag=f"th{h}")
            nc.scalar.activation(th, Cs[h], AF.Tanh, bias=zbias, scale=0.5)
            nc.vector.scalar_tensor_tensor(
                buf2[0:96, col2 + h * BH:col2 + (h + 1) * BH],
                to, 1.0, th, ALU.add, ALU.mult,
            )

    # FC head: final h2 = H2/2 lives in rows 64:96 of block T+1
    ckf, ttf = divmod(T + 1, TC)
    final = strips[ckf % 2]
    colf = ttf * BL
    for h in range(NH):
        fcin = work.tile([33, BH], BF16, name="fcin", tag="fcin", bufs=2)
        nc.vector.tensor_copy(
            out=fcin[0:32, :], in_=final[64:96, colf + h * BH:colf + (h + 1) * BH]
        )
        nc.any.memset(fcin[32:33, :], 1.0)
        fps = psum.tile([F1, BH], F32, name="fps", tag="fps", bufs=1)
        nc.tensor.matmul(fps, wfc1_sb, fcin, start=True, stop=True)
        rr = work.tile([33, BH], BF16, name="rr", tag="rr")
        nc.any.memset(rr[0:33, :], 0.0)
        nc.any.memset(rr[32:33, :], 1.0)
        nc.scalar.activation(rr[0:F1, :], fps, AF.Relu)
        ops = psum.tile([OUT, BH], F32, name="ops", tag="ops", bufs=1)
        nc.tensor.matmul(ops, wfc2_sb, rr, start=True, stop=True)
        nc.vector.tensor_copy(out=out_sb[:, h * BH:(h + 1) * BH], in_=ops)
    nc.sync.dma_start(out=out, in_=out_sb)


def build_program(mw=128, f16=True):
    nc = bacc.Bacc(
        "TRN2",
        target_bir_lowering=False,
        debug=False,
        num_devices=NCORES,
    )
    x_d = nc.dram_tensor("x", [IN + 1, T, BL], BF16, kind="ExternalInput")
    w_d = nc.dram_tensor("w", [KP, 4 * mw], BF16, kind="ExternalInput")
    w0_d = nc.dram_tensor("w0", [KP, 4 * mw], BF16, kind="ExternalInput")
    wfc1_d = nc.dram_tensor("wfc1", [33, F1], BF16, kind="ExternalInput")
    wfc2_d = nc.dram_tensor("wfc2", [33, OUT], BF16, kind="ExternalInput")
    out_d = nc.dram_tensor("out", [OUT, BL], F32, kind="ExternalOutput")

    with tile.TileContext(nc) as tc_, ExitStack() as ctx:
        _build_body(
            ctx, tc_, x_d.ap(), w_d.ap(), w0_d.ap(), wfc1_d.ap(), wfc2_d.ap(),
            out_d.ap(), mw=mw, f16=f16,
        )
    nc.compile()
    return nc


def pack_weights(inp, mw=128):
    """Pack LSTM+FC weights into the fused bf16 layout (see module docstring)."""
    s = {"i": 0.5, "f": 0.5, "o": 0.5, "g": 1.0}

    def rows(q, H):
        idx = {"i": 0, "f": 1, "g": 2, "o": 3}[q]  # pytorch gate order
        return slice(idx * H, (idx + 1) * H)

    # strip rows: 0:64 H1, 64:96 H2, 96:122 x, 122 ones
    W = np.zeros((KP, 4 * mw), np.float32)
    for gi, q in enumerate(["i", "f", "o", "g"]):
        c0 = gi * mw
        r1 = rows(q, H1)
        W[96:122, c0:c0 + 64] = s[q] * inp["w_ih1"][r1].T
        W[122, c0:c0 + 64] = s[q] * (inp["b_ih1"][r1] + inp["b_hh1"][r1])
        W[0:64, c0:c0 + 64] = s[q] * 0.5 * inp["w_hh1"][r1].T
        r2 = rows(q, H2)
        W[0:64, c0 + 64:c0 + 96] = s[q] * 0.5 * inp["w_ih2"][r2].T
        W[64:96, c0 + 64:c0 + 96] = s[q] * 0.5 * inp["w_hh2"][r2].T
        W[122, c0 + 64:c0 + 96] = s[q] * (inp["b_ih2"][r2] + inp["b_hh2"][r2])
    W0 = W.copy()
    for gi in range(4):
        W0[:, gi * mw + 64:gi * mw + 96] = 0.0

    fc1 = np.zeros((33, F1), np.float32)
    fc1[0:32] = 0.5 * inp["w_fc1"].T
    fc1[32] = inp["b_fc1"]
    fc2 = np.zeros((33, OUT), np.float32)
    fc2[0:F1] = inp["w_fc2"].T
    fc2[32] = inp["b_fc2"]
    cast = lambda a: a.astype(ml_dtypes.bfloat16)
    return cast(W), cast(W0), cast(fc1), cast(fc2)


_NC_CACHE = None


def _cfg():
    mw = int(os.environ.get("KERNEL_MW", 128))
    f16 = os.environ.get("KERNEL_F16", "1") == "1"
    return mw, f16


def get_program():
    global _NC_CACHE
    if _NC_CACHE is None:
        mw, f16 = _cfg()
        _NC_CACHE = build_program(mw=mw, f16=f16)
    return _NC_CACHE


def _make_in_maps(inp):
    mw, _ = _cfg()
    W, W0, fc1, fc2 = pack_weights(inp, mw=mw)
    xc = np.asarray(inp["x"][:, 0])  # [512, 26, 1000] fp32
    in_maps = []
    for c in range(NCORES):
        # [BL, 26, T] -> [26, T, BL], append ones row -> [27, T, BL], bf16
        xs = np.transpose(xc[c * BL:(c + 1) * BL], (1, 2, 0))
        xp = np.concatenate([xs, np.ones((1, T, BL), np.float32)], axis=0)
        in_maps.append({
            "x": np.ascontiguousarray(xp).astype(ml_dtypes.bfloat16),
            "w": W,
            "w0": W0,
            "wfc1": fc1,
            "wfc2": fc2,
        })
    return in_maps


def kernel(**inputs):
    inp = {k: np.asarray(v) for k, v in inputs.items()}
    in_maps = _make_in_maps(inp)
    nc = get_program()
    res = bass_utils.run_bass_kernel_spmd(nc, in_maps, core_ids=list(range(NCORES)))
    outs = [np.asarray(res.results[c]["out"], np.float32) for c in range(NCORES)]
    return np.concatenate([o.T for o in outs], axis=0).astype(np.float32)


if __name__ == "__main__":
    rng = np.random.default_rng(0)
    fake = {
        "x": rng.standard_normal((B, 1, IN, T), dtype=np.float32),
        "w_ih1": rng.standard_normal((4 * H1, IN), dtype=np.float32) * 0.1,
        "w_hh1": rng.standard_normal((4 * H1, H1), dtype=np.float32) * 0.1,
        "b_ih1": rng.standard_normal(4 * H1).astype(np.float32) * 0.1,
        "b_hh1": rng.standard_normal(4 * H1).astype(np.float32) * 0.1,
        "w_ih2": rng.standard_normal((4 * H2, H1), dtype=np.float32) * 0.1,
        "w_hh2": rng.standard_normal((4 * H2, H2), dtype=np.float32) * 0.1,
        "b_ih2": rng.standard_normal(4 * H2).astype(np.float32) * 0.1,
        "b_hh2": rng.standard_normal(4 * H2).astype(np.float32) * 0.1,
        "w_fc1": rng.standard_normal((F1, H2), dtype=np.float32) * 0.1,
        "b_fc1": rng.standard_normal(F1).astype(np.float32) * 0.1,
        "w_fc2": rng.standard_normal((OUT, F1), dtype=np.float32) * 0.1,
        "b_fc2": rng.standard_normal(OUT).astype(np.float32) * 0.1,
    }
    y = kernel(**fake)
    print("kernel output", y.shape, y.dtype, np.abs(y).max())


# revision 12
# speedup vs baseline: 1.0015x; 1.0015x over previous
"""Trainium2 Bass kernel for nn_AudioLSTM (2-layer LSTM + 2-layer FC head).

Strategy (per core; pure data parallelism over batch, 8 cores x 64 batch):
  - Two independent batch halves of 32 (software pipelining of the serial
    T=1000 recurrence across engines).
  - Sliding-window "strip" state layout: strip[123, 250*64] bf16 per chunk
    (double buffered).  Rows 0:96 = [H1(64); H2(32)] (H = 2*h), rows
    96:122 = x_t, row 122 = ones.  Block t (cols t*64:(t+1)*64) is the
    matmul rhs for step t; the H update of step t writes directly into
    block t+1, and x (+ ones row) is staged by one contiguous DMA per
    250-step chunk from a host-side pre-transposed [27, T, 64] bf16 tensor.
    => zero per-step data-movement ops.
  - 4 matmuls per step per half (one per gate q in {i,f,o,g}), each
    lhsT [123, MW] -> psum [MW, 32]: rows 0:64 = LSTM1 gate, 64:96 = LSTM2
    gate (MW=128 pads gate blocks with zero cols to trigger FWL).
    LSTM2 runs one step behind LSTM1 so both layers read the same block.
  - tanh-everywhere: sigma(z) = (1+tanh(z/2))/2; the 1/2 scales are folded
    into the packed weights, so ONE Tanh activation covers all 4 gates.
  - Cell state C = 2*c in fp32:  A=(tf+1)*C; B=(ti+1)*tg; C=0.5*A+B;
    th=tanh(0.5*C); H=(to+1)*th   (4 DVE scalar_tensor_tensor ops).
  - Iteration 0 uses a weight copy with the LSTM2 columns zeroed, which makes
    the one-step-behind LSTM2 start exactly from h2=c2=0.  One extra
    iteration (t=T) lets LSTM2 finish its last step.
"""
import os
import sys
from contextlib import ExitStack

import numpy as np

sys.path.insert(0, "/opt/trn_rl_repo")

import ml_dtypes

import concourse.bacc as bacc
import concourse.mybir as mybir
from concourse import bass_utils, tile

AF = mybir.ActivationFunctionType
ALU = mybir.AluOpType
BF16 = mybir.dt.bfloat16
F16 = mybir.dt.float16
F32 = mybir.dt.float32

IN, H1, H2, F1, OUT = 26, 64, 32, 16, 10
B, T = 512, 1000
NCORES = 8
BL = B // NCORES          # 64 batch per core
NH = 2                    # batch halves per core (software pipelining)
BH = BL // NH             # 32
TC = 250                  # time chunk for x staging
NCK = T // TC
KP = 123                  # state rows: 96 H + 26 x + 1 ones


def _build_body(ctx: ExitStack, tc_: tile.TileContext, x, w, w0, wfc1, wfc2, out,
                mw=128, f16=True):
    nc = tc_.nc
    DT = F16 if f16 else F32

    const = ctx.enter_context(tc_.tile_pool(name="const", bufs=1))
    psum = ctx.enter_context(tc_.tile_pool(name="ps", bufs=3, space="PSUM"))
    work = ctx.enter_context(tc_.tile_pool(name="wk", bufs=4))

    w_sb = const.tile([KP, 4 * mw], BF16)
    nc.sync.dma_start(out=w_sb, in_=w)
    w0_sb = const.tile([KP, 4 * mw], BF16)
    nc.sync.dma_start(out=w0_sb, in_=w0)
    wfc1_sb = const.tile([33, F1], BF16)
    nc.sync.dma_start(out=wfc1_sb, in_=wfc1)
    wfc2_sb = const.tile([33, OUT], BF16)
    nc.sync.dma_start(out=wfc2_sb, in_=wfc2)

    # Pre-warm the ACT tanh table during startup DMAs so the first real TANH
    # doesn't pay the ~2.7us ACT_TABLE_LOAD on the critical path.
    warm = work.tile([1, 1], F32)
    nc.any.memset(warm, 0.0)
    wout = work.tile([1, 1], F32)
    nc.scalar.activation(wout, warm, AF.Tanh)

    strips = [const.tile([KP, BL * TC], BF16, name=f"strip{i}") for i in range(2)]
    # chunk 0 x (+ones row) staging; split so the first steps' x lands fast
    # and the recurrence starts without waiting on the full 864KB transfer.
    PRE = 16
    nc.sync.dma_start(out=strips[0][96:123, 0:PRE * BL], in_=x[:, 0:PRE, :])
    nc.sync.dma_start(out=strips[0][96:123, PRE * BL:], in_=x[:, PRE:TC, :])
    nc.vector.memset(strips[0][0:96, 0:BL], 0.0)
    # Cell-state tiles live in the work-pool arena (bufs=1 -> persistent slot)
    # so the per-step DVE ops touching them stay within one SBUF neighborhood;
    # const-pool placement behind the 62KB strips measured ~80ns/op slower.
    Cs = []
    for h in range(NH):
        C_h = work.tile([96, BH], DT, name=f"C{h}", tag=f"C{h}", bufs=1)
        nc.any.memset(C_h, 0.0)
        Cs.append(C_h)
    # Near-placed zero bias vector for the activations (the implicit const-AP
    # bias lives in a far SBUF region).
    zbias = work.tile([96, 1], F32, name="zbias", tag="zbias", bufs=1)
    nc.any.memset(zbias, 0.0)

    out_sb = const.tile([OUT, BL], F32)

    for t in range(T + 1):
        ck, tt = divmod(t, TC)
        buf = strips[ck % 2]
        col = tt * BL
        if tt == 0 and ck + 1 < NCK:
            nxt = strips[(ck + 1) % 2]
            nc.sync.dma_start(
                out=nxt[96:123, :], in_=x[:, (ck + 1) * TC:(ck + 2) * TC, :]
            )
        ck2, tt2 = divmod(t + 1, TC)
        buf2 = strips[ck2 % 2]
        col2 = tt2 * BL
        wsel = w0_sb if t == 0 else w_sb
        # Emission order is engine-queue order (in-order engines).  Interleave
        # the two halves' chains so ACT runs TANH(h0), TANH(h1), T05(h0),
        # T05(h1) per step instead of serializing each half's full chain.
        ASs = []
        for h in range(NH):
            rhs = buf[:, col + h * BH:col + (h + 1) * BH]
            ps = psum.tile([mw, 4 * BH], F32, name="ps", tag=f"ps{h}")
            for gi in range(4):
                nc.tensor.matmul(
                    ps[:, gi * BH:(gi + 1) * BH],
                    wsel[:, gi * mw:(gi + 1) * mw],
                    rhs,
                    start=True,
                    stop=True,
                )
            AS = work.tile([96, 4 * BH], DT, name="AS", tag=f"AS{h}")
            nc.scalar.activation(AS, ps[0:96, :], AF.Tanh, bias=zbias)
            ASs.append(AS)
        # Interleave both halves' product ops before the C updates: the DVE
        # stalls ~100ns when an op reads the immediately-preceding op's output
        # (pipe-drain RAW); with Bv/Av of both halves first, each C's inputs
        # are >=2 ops back and the drain is hidden.
        AvBv = []
        for h in range(NH):
            AS = ASs[h]
            ti = AS[:, 0:BH]
            tf = AS[:, BH:2 * BH]
            tg = AS[:, 3 * BH:4 * BH]
            Bv = work.tile([96, BH], DT, name="Bv", tag=f"Bv{h}")
            nc.vector.scalar_tensor_tensor(Bv, ti, 1.0, tg, ALU.add, ALU.mult)
            Av = work.tile([96, BH], DT, name="Av", tag=f"Av{h}")
            nc.vector.scalar_tensor_tensor(Av, tf, 1.0, Cs[h], ALU.add, ALU.mult)
            AvBv.append((Av, Bv))
        for h in range(NH):
            Av, Bv = AvBv[h]
            nc.vector.affine_then_add(Cs[h], Av, Bv, 0.5, 0.0)
        for h in range(NH):
            to = ASs[h][:, 2 * BH:3 * BH]
            th = work.tile([96, BH], DT, name="th", tag=f"th{h}")
            nc.scalar.activation(th, Cs[h], AF.Tanh, bias=zbias, scale=0.5)
            nc.vector.scalar_tensor_tensor(
                buf2[0:96, col2 + h * BH:col2 + (h + 1) * BH],
                to, 1.0, th, ALU.add, ALU.mult,
            )

    # FC head: final h2 = H2/2 lives in rows 64:96 of block T+1
    ckf, ttf = divmod(T + 1, TC)
    final = strips[ckf % 2]
    colf = ttf * BL
    for h in range(NH):
        fcin = work.tile([33, BH], BF16, name="fcin", tag="fcin", bufs=2)
        nc.vector.tensor_copy(
            out=fcin[0:32, :], in_=final[64:96, colf + h * BH:colf + (h + 1) * BH]
        )
        nc.any.memset(fcin[32:33, :], 1.0)
        fps = psum.tile([F1, BH], F32, name="fps", tag="fps", bufs=1)
        nc.tensor.matmul(fps, wfc1_sb, fcin, start=True, stop=True)
        rr = work.tile([33, BH], BF16, name="rr", tag="rr")
        nc.any.memset(rr[0:33, :], 0.0)
        nc.any.memset(rr[32:33, :], 1.0)
        nc.scalar.activation(rr[0:F1, :], fps, AF.Relu)
        ops = psum.tile([OUT, BH], F32, name="ops", tag="ops", bufs=1)
        nc.tensor.matmul(ops, wfc2_sb, rr, start=True, stop=True)
        nc.vector.tensor_copy(out=out_sb[:, h * BH:(h + 1) * BH], in_=ops)
    nc.sync.dma_start(out=out, in_=out_sb)


def build_program(mw=128, f16=True):
    nc = bacc.Bacc(
        "TRN2",
        target_bir_lowering=False,
        debug=False,
        num_devices=NCORES,
    )
    x_d = nc.dram_tensor("x", [IN + 1, T, BL], BF16, kind="ExternalInput")
    w_d = nc.dram_tensor("w", [KP, 4 * mw], BF16, kind="ExternalInput")
    w0_d = nc.dram_tensor("w0", [KP, 4 * mw], BF16, kind="ExternalInput")
    wfc1_d = nc.dram_tensor("wfc1", [33, F1], BF16, kind="ExternalInput")
    wfc2_d = nc.dram_tensor("wfc2", [33, OUT], BF16, kind="ExternalInput")
    out_d = nc.dram_tensor("out", [OUT, BL], F32, kind="ExternalOutput")

    with tile.TileContext(nc) as tc_, ExitStack() as ctx:
        _build_body(
            ctx, tc_, x_d.ap(), w_d.ap(), w0_d.ap(), wfc1_d.ap(), wfc2_d.ap(),
            out_d.ap(), mw=mw, f16=f16,
        )
    nc.compile()
    return nc


def pack_weights(inp, mw=128):
    """Pack LSTM+FC weights into the fused bf16 layout (see module docstring)."""
    s = {"i": 0.5, "f": 0.5, "o": 0.5, "g": 1.0}

    def rows(q, H):
        idx = {"i": 0, "f": 1, "g": 2, "o": 3}[q]  # pytorch gate order
        return slice(idx * H, (idx + 1) * H)

    # strip rows: 0:64 H1, 64:96 H2, 96:122 x, 122 ones
    W = np.zeros((KP, 4 * mw), np.float32)
    for gi, q in enumerate(["i", "f", "o", "g"]):
        c0 = gi * mw
        r1 = rows(q, H1)
        W[96:122, c0:c0 + 64] = s[q] * inp["w_ih1"][r1].T
        W[122, c0:c0 + 64] = s[q] * (inp["b_ih1"][r1] + inp["b_hh1"][r1])
        W[0:64, c0:c0 + 64] = s[q] * 0.5 * inp["w_hh1"][r1].T
        r2 = rows(q, H2)
        W[0:64, c0 + 64:c0 + 96] = s[q] * 0.5 * inp["w_ih2"][r2].T
        W[64:96, c0 + 64:c0 + 96] = s[q] * 0.5 * inp["w_hh2"][r2].T
        W[122, c0 + 64:c0 + 96] = s[q] * (inp["b_ih2"][r2] + inp["b_hh2"][r2])
    W0 = W.copy()
    for gi in range(4):
        W0[:, gi * mw + 64:gi * mw + 96] = 0.0

    fc1 = np.zeros((33, F1), np.float32)
    fc1[0:32] = 0.5 * inp["w_fc1"].T
    fc1[32] = inp["b_fc1"]
    fc2 = np.zeros((33, OUT), np.float32)
    fc2[0:F1] = inp["w_fc2"].T
    fc2[32] = inp["b_fc2"]
    cast = lambda a: a.astype(ml_dtypes.bfloat16)
    return cast(W), cast(W0), cast(fc1), cast(fc2)


_NC_CACHE = None


def _cfg():
    mw = int(os.environ.get("KERNEL_MW", 128))
    f16 = os.environ.get("KERNEL_F16", "1") == "1"
    return mw, f16


def get_program():
    global _NC_CACHE
    if _NC_CACHE is None:
        mw, f16 = _cfg()
        _NC_CACHE = build_program(mw=mw, f16=f16)
    return _NC_CACHE


def _make_in_maps(inp):
    mw, _ = _cfg()
    W, W0, fc1, fc2 = pack_weights(inp, mw=mw)
    xc = np.asarray(inp["x"][:, 0])  # [512, 26, 1000] fp32
    in_maps = []
    for c in range(NCORES):
        # [BL, 26, T] -> [26, T, BL], append ones row -> [27, T, BL], bf16
        xs = np.transpose(xc[c * BL:(c + 1) * BL], (1, 2, 0))
        xp = np.concatenate([xs, np.ones((1, T, BL), np.float32)], axis=0)
        in_maps.append({
            "x": np.ascontiguousarray(xp).astype(ml_dtypes.bfloat16),
            "w": W,
            "w0": W0,
            "wfc1": fc1,
            "wfc2": fc2,
        })
    return in_maps


def kernel(**inputs):
    inp = {k: np.asarray(v) for k, v in inputs.items()}
    in_maps = _make_in_maps(inp)
    nc = get_program()
    res = bass_utils.run_bass_kernel_spmd(nc, in_maps, core_ids=list(range(NCORES)))
    outs = [np.asarray(res.results[c]["out"], np.float32) for c in range(NCORES)]
    return np.concatenate([o.T for o in outs], axis=0).astype(np.float32)


if __name__ == "__main__":
    rng = np.random.default_rng(0)
    fake = {
        "x": rng.standard_normal((B, 1, IN, T), dtype=np.float32),
        "w_ih1": rng.standard_normal((4 * H1, IN), dtype=np.float32) * 0.1,
        "w_hh1": rng.standard_normal((4 * H1, H1), dtype=np.float32) * 0.1,
        "b_ih1": rng.standard_normal(4 * H1).astype(np.float32) * 0.1,
        "b_hh1": rng.standard_normal(4 * H1).astype(np.float32) * 0.1,
        "w_ih2": rng.standard_normal((4 * H2, H1), dtype=np.float32) * 0.1,
        "w_hh2": rng.standard_normal((4 * H2, H2), dtype=np.float32) * 0.1,
        "b_ih2": rng.standard_normal(4 * H2).astype(np.float32) * 0.1,
        "b_hh2": rng.standard_normal(4 * H2).astype(np.float32) * 0.1,
        "w_fc1": rng.standard_normal((F1, H2), dtype=np.float32) * 0.1,
        "b_fc1": rng.standard_normal(F1).astype(np.float32) * 0.1,
        "w_fc2": rng.standard_normal((OUT, F1), dtype=np.float32) * 0.1,
        "b_fc2": rng.standard_normal(OUT).astype(np.float32) * 0.1,
    }
    y = kernel(**fake)
    print("kernel output", y.shape, y.dtype, np.abs(y).max())
